# revision 1
# baseline (speedup 1.0000x reference)
"""Causal multi-head attention on 8 Trainium2 NeuronCores.

Sharding: Megatron-style tensor parallelism over heads. Each of the 8
cores computes 2 of the 16 heads end-to-end:
  - column-parallel Q/K/V projections (each core owns a 256-feature slice
    of wq/wk/wv),
  - per-head causal attention entirely on-core,
  - row-parallel output projection producing a partial [B*S, D] output.
The 8 partials are summed on the host (the "all-reduce") and bo added.

Device layout notes:
  - All matmul operands are bf16 (the PE streams bf16 moving operands at
    ~2x the fp32/fp32r rate); accumulation is fp32 in PSUM, softmax
    statistics and the partial outputs stay fp32.
  - Activations are kept feature-major (transposed): qT/kT are [hd, S]
    per head, scores are computed transposed ([k, q]) so the exp'd
    probabilities feed the PV matmul directly as the moving operand and
    the softmax denominator comes from a ones-matmul — the kernel
    contains zero on-chip transposes.
  - Causality is exploited at block granularity: upper-triangle score
    blocks are never computed; diagonal blocks get a static additive
    mask; partially-causal blocks are column-trimmed at 128 granularity.
"""

import math

import numpy as np

B = 2
S = 2048
D = 2048
H = 16
HD = 128  # head dim
N_CORES = 8
H_LOC = H // N_CORES       # 2 heads per core
F_LOC = H_LOC * HD         # 256 local features per core
KT = D // 128              # 16 contraction tiles
CHUNK = 512                # token chunk (matmul moving dim)
NCH = S // CHUNK           # 4 chunks per batch
TT = S // 128              # 16 token tiles per batch

_CACHE = {}


MM_DTYPE = "bf16"  # "bf16" or "f32r" — dtype of all matmul operands
STRUCTURE = "v1"   # "v1": per-batch QKV->attn+outproj; "v2": cross-batch pipeline


def _build(reps=None, mm_dtype=None):
    import concourse.mybir as mybir
    import concourse.tile as tile
    from concourse import bacc

    F32 = mybir.dt.float32
    # MDT is the dtype of every matmul operand (bf16 by default: the PE
    # streams bf16 at 2 cols/cycle vs fp32r's effective half rate).
    MDT = (mybir.dt.bfloat16 if (mm_dtype or MM_DTYPE) == "bf16"
           else mybir.dt.float32r)
    ADD = mybir.AluOpType.add
    MULT = mybir.AluOpType.mult
    EXP = mybir.ActivationFunctionType.Exp
    INV_SQRT_HD = 1.0 / math.sqrt(HD)

    nc = bacc.Bacc("TRN2", target_bir_lowering=False, debug=False,
                   num_devices=N_CORES)

    xT_d = nc.dram_tensor("xT", [D, B * S], MDT, kind="ExternalInput")
    wqT_d = nc.dram_tensor("wqT", [D, F_LOC], MDT, kind="ExternalInput")
    wkT_d = nc.dram_tensor("wkT", [D, F_LOC], MDT, kind="ExternalInput")
    wvT_d = nc.dram_tensor("wvT", [D, F_LOC], MDT, kind="ExternalInput")
    woT_d = nc.dram_tensor("woT", [F_LOC, D], MDT, kind="ExternalInput")
    bq_d = nc.dram_tensor("bq2", [HD, H_LOC], F32, kind="ExternalInput")
    bk_d = nc.dram_tensor("bk2", [HD, H_LOC], F32, kind="ExternalInput")
    bv_d = nc.dram_tensor("bvr", [128, F_LOC], F32, kind="ExternalInput")
    ones_d = nc.dram_tensor("ones", [128, 128], MDT, kind="ExternalInput")
    y_d = nc.dram_tensor("y", [B * S, D], F32, kind="ExternalOutput")

    with tile.TileContext(nc) as tc:
        cpool = tc.alloc_tile_pool(name="const", bufs=1)
        wpool = tc.alloc_tile_pool(name="w", bufs=1)
        xkpool = tc.alloc_tile_pool(name="xk", bufs=4)
        actpool = tc.alloc_tile_pool(name="act", bufs=8)
        ypool = tc.alloc_tile_pool(name="y", bufs=4)
        ripool = tc.alloc_tile_pool(name="ri", bufs=2)
        # one deep shared accumulator pool (QKV / scores / PV / rowsum)
        # plus a small dedicated pool for the output projection
        psq = tc.alloc_tile_pool(name="ps", bufs=5, space="PSUM")
        psa = psr = psq
        pso = tc.alloc_tile_pool(name="pso", bufs=3, space="PSUM")
        qtag = "ps"

        # --- loads; DMA queue order is deliberate (wq + first x chunks
        # first so the projection matmuls start a few us in) ---
        def load_w(nm, dram):
            w_t = wpool.tile([128, KT * F_LOC], MDT, tag=nm)
            nc.sync.dma_start(
                w_t[:].rearrange("p (k f) -> p k f", k=KT),
                dram.ap().rearrange("(k p) f -> p k f", p=128),
            )
            return w_t

        def load_x(b, c, split=1):
            x_t = xkpool.tile([128, KT * CHUNK], MDT, tag="xk")
            col0 = b * S + c * CHUNK
            kstep = KT // split
            for s in range(split):
                k0 = s * kstep
                nc.sync.dma_start(
                    x_t[:, k0 * CHUNK:(k0 + kstep) * CHUNK]
                        .rearrange("p (k f) -> p k f", k=kstep),
                    xT_d.ap()[k0 * 128:(k0 + kstep) * 128,
                              col0:col0 + CHUNK]
                        .rearrange("(k p) f -> p k f", p=128),
                )
            return x_t

        w_ts = {"wq": load_w("wq", wqT_d)}
        x_first = load_x(0, 0, split=4)
        w_ts["wk"] = load_w("wk", wkT_d)
        w_ts["wv"] = load_w("wv", wvT_d)
        x_second = load_x(0, 1, split=2)
        bq_t = cpool.tile([HD, H_LOC], F32, tag="bq")
        bk_t = cpool.tile([HD, H_LOC], F32, tag="bk")
        bv_t = cpool.tile([128, F_LOC], F32, tag="bv")
        nc.sync.dma_start(bq_t[:], bq_d.ap())
        nc.sync.dma_start(bk_t[:], bk_d.ap())
        nc.sync.dma_start(bv_t[:], bv_d.ap())
        ones128 = cpool.tile([128, 128], MDT, tag="ones128")
        nc.sync.dma_start(ones128[:], ones_d.ap())
        # warm the ACT Exp table during the QKV phase so the table load
        # doesn't land on the first attention chunk
        warm_t = cpool.tile([128, 1], F32, tag="warm")
        nc.scalar.activation(warm_t[:], bq_t[:, 0:1], EXP,
                             bias=0.0, scale=1.0)
        maskT = cpool.tile([128, 128], F32, tag="maskT")
        nc.gpsimd.memset(maskT[:], 0.0)
        # transposed causal mask: keep (0) where k_part <= q_free else -1e9
        nc.gpsimd.affine_select(
            out=maskT[:], in_=maskT[:],
            compare_op=mybir.AluOpType.is_ge,
            fill=-1e9, base=0, pattern=[[1, 128]], channel_multiplier=-1,
        )

        def qkv_chunk(x_t, c, acts):
            """Q/K/V projections for one 512-token chunk."""
            qT_t, kT_t, v_t, _ = acts
            for nm, dst, bias in (("wq", qT_t, bq_t), ("wk", kT_t, bk_t)):
                for h in range(H_LOC):
                    q_ps = psq.tile([128, CHUNK], F32, tag=qtag)
                    for k in range(KT):
                        nc.tensor.matmul(
                            q_ps[:],
                            w_ts[nm][:, k * F_LOC + h * HD:
                                     k * F_LOC + (h + 1) * HD],
                            x_t[:, k * CHUNK:(k + 1) * CHUNK],
                            start=(k == 0), stop=(k == KT - 1),
                        )
                    nc.vector.tensor_scalar_add(
                        dst[:, h * S + c * CHUNK: h * S + (c + 1) * CHUNK],
                        q_ps[:], bias[:, h:h + 1])
            for t4 in range(CHUNK // 128):
                tt = c * (CHUNK // 128) + t4
                v_ps = psq.tile([128, CHUNK], F32, tag=qtag)
                for k in range(KT):
                    nc.tensor.matmul(
                        v_ps[:, 0:F_LOC],
                        x_t[:, k * CHUNK + t4 * 128:
                            k * CHUNK + (t4 + 1) * 128],
                        w_ts["wv"][:, k * F_LOC:(k + 1) * F_LOC],
                        start=(k == 0), stop=(k == KT - 1),
                    )
                # bias folded into the PSUM->SBUF copy (bv broadcast across
                # partitions, prepared on the host)
                nc.vector.tensor_tensor(
                    v_t[:, tt * F_LOC:(tt + 1) * F_LOC],
                    v_ps[:, 0:F_LOC], bv_t[:], ADD)

        def attn_chunk(c, acts):
            """Causal attention for one 512-query chunk, both heads."""
            qT_t, kT_t, v_t, attnT_t = acts
            nki = 4 * c + 4
            for h in range(H_LOC):
                e_t = xkpool.tile([128, KT * CHUNK], MDT, tag="xk")
                q0 = h * S + c * CHUNK
                # scoresT blocks + exp (transposed layout: [k, q])
                for ki in range(nki):
                    r = ki - 4 * c
                    trim = 128 * r if r > 0 else 0
                    ncol = CHUNK - trim
                    s_ps = psa.tile([128, CHUNK], F32, tag=qtag)
                    nc.tensor.matmul(
                        s_ps[:, 0:ncol],
                        kT_t[:, h * S + ki * 128: h * S + (ki + 1) * 128],
                        qT_t[:, q0 + trim: q0 + CHUNK],
                        start=True, stop=True,
                    )
                    if ki >= 4 * c:  # diagonal 128x128 needs the mask
                        nc.vector.tensor_tensor(
                            s_ps[:, 0:128], s_ps[:, 0:128], maskT[:], ADD)
                    nc.scalar.activation(
                        e_t[:, ki * CHUNK + trim:(ki + 1) * CHUNK],
                        s_ps[:, 0:ncol], EXP, bias=0.0, scale=INV_SQRT_HD)
                # PV and rowsum accumulations over ki (PE)
                at_ps = psr.tile([128, CHUNK], F32, tag=qtag)
                rs_ps = psr.tile([128, CHUNK], F32, tag=qtag)
                for ki in range(nki):
                    r = ki - 4 * c
                    trim = 128 * r if r > 0 else 0
                    nc.tensor.matmul(
                        at_ps[:, trim:CHUNK],
                        v_t[:, ki * F_LOC + h * HD:
                            ki * F_LOC + (h + 1) * HD],
                        e_t[:, ki * CHUNK + trim:(ki + 1) * CHUNK],
                        start=(ki == 0), stop=(ki == nki - 1),
                    )
                for ki in range(nki):
                    r = ki - 4 * c
                    trim = 128 * r if r > 0 else 0
                    nc.tensor.matmul(
                        rs_ps[:, trim:CHUNK],
                        ones128[:],
                        e_t[:, ki * CHUNK + trim:(ki + 1) * CHUNK],
                        start=(ki == 0), stop=(ki == nki - 1),
                    )
                ri_t = ripool.tile([128, CHUNK], F32, tag="ri")
                nc.vector.reciprocal(ri_t[:], rs_ps[:])
                nc.vector.tensor_tensor(
                    attnT_t[:, q0: q0 + CHUNK],
                    at_ps[:], ri_t[:], MULT)

        def outproj_chunk(b, c, acts, wo_t):
            """Output projection + y writeback for one chunk's tokens."""
            attnT_t = acts[3]
            for t4 in range(CHUNK // 128):
                tt = c * (CHUNK // 128) + t4
                for oc in range(D // CHUNK):
                    o_ps = pso.tile([128, CHUNK], F32, tag="pso")
                    for h in range(H_LOC):
                        nc.tensor.matmul(
                            o_ps[:],
                            attnT_t[:, h * S + tt * 128:
                                    h * S + (tt + 1) * 128],
                            wo_t[:, h * D + oc * CHUNK:
                                 h * D + (oc + 1) * CHUNK],
                            start=(h == 0), stop=(h == H_LOC - 1),
                        )
                    y_t = ypool.tile([128, CHUNK], F32, tag="y")
                    nc.vector.tensor_copy(y_t[:], o_ps[:])
                    row0 = b * S + tt * 128
                    nc.sync.dma_start(
                        y_d.ap()[row0:row0 + 128,
                                 oc * CHUNK:(oc + 1) * CHUNK], y_t[:])

        def new_acts():
            qT_t = actpool.tile([128, H_LOC * S], MDT, tag="act")
            kT_t = actpool.tile([128, H_LOC * S], MDT, tag="act")
            v_t = actpool.tile([128, TT * F_LOC], MDT, tag="act")
            attnT_t = actpool.tile([128, H_LOC * S], MDT, tag="act")
            return (qT_t, kT_t, v_t, attnT_t)

        def load_wo():
            # woT [F_LOC, D] -> [128, H_LOC*D]; deferred load so the DMA
            # queue prioritizes x chunks during warmup
            wo_t = wpool.tile([128, H_LOC * D], MDT, tag="wo")
            nc.sync.dma_start(
                wo_t[:].rearrange("p (h f) -> p h f", h=H_LOC),
                woT_d.ap().rearrange("(h p) f -> p h f", p=128),
            )
            return wo_t

        def emit_body_v2(first_iter=True):
            # phase 1: QKV(b0) — PE-bound, streams x(b0)
            acts0 = new_acts()
            for c in range(NCH):
                if first_iter and c == 0:
                    x_t = x_first
                elif first_iter and c == 1:
                    x_t = x_second
                else:
                    x_t = load_x(0, c, split=(4 if c == 0 else 1))
                qkv_chunk(x_t, c, acts0)

            wo_t = load_wo()

            # phase 2: QKV(b1) [PE-bound] interleaved with attention(b0)
            # [ACT-bound] + outproj(b0) [DVE/DMA-bound]
            acts1 = new_acts()
            for c in range(NCH):
                qkv_chunk(load_x(1, c), c, acts1)
                attn_chunk(c, acts0)
                outproj_chunk(0, c, acts0, wo_t)

            # phase 3: attention(b1) + outproj(b1)
            for c in range(NCH):
                attn_chunk(c, acts1)
                outproj_chunk(1, c, acts1, wo_t)

        def emit_body_v1(first_iter=True):
            wo_t = None
            for b in range(B):
                acts = new_acts()
                for c in range(NCH):
                    if first_iter and b == 0 and c == 0:
                        x_t = x_first
                    elif first_iter and b == 0 and c == 1:
                        x_t = x_second
                    else:
                        x_t = load_x(b, c,
                                     split=(4 if (b == 0 and c == 0) else 1))
                    if wo_t is None and c == NCH - 1:
                        wo_t = load_wo()
                    qkv_chunk(x_t, c, acts)
                for c in range(NCH):
                    attn_chunk(c, acts)
                    outproj_chunk(b, c, acts, wo_t)

        emit_body = emit_body_v2 if STRUCTURE == "v2" else emit_body_v1

        if reps is None:
            emit_body()
        else:
            with tc.For_i(0, reps, 1):
                emit_body(first_iter=False)

        pools = [pso, psq, ripool, ypool, actpool, xkpool,
                 wpool, cpool]
        seen = set()
        for p in pools:
            if id(p) not in seen:
                seen.add(id(p))
                p.release()

    nc.compile()
    return nc


def _get_nc(reps=None, mm_dtype=None):
    key = ("nc", reps, mm_dtype or MM_DTYPE)
    if key not in _CACHE:
        _CACHE[key] = _build(reps, mm_dtype)
    return _CACHE[key]


def _mm_np(a):
    """Cast a host array to the matmul operand dtype."""
    if MM_DTYPE == "bf16":
        import ml_dtypes
        return np.ascontiguousarray(a).astype(ml_dtypes.bfloat16)
    return np.ascontiguousarray(a).astype(np.float32)


def make_in_maps(x, wq, bq, wk, bk, wv, bv, wo):
    x = np.asarray(x, dtype=np.float32)
    xT = _mm_np(x.reshape(B * S, D).T)  # [D, B*S]

    in_maps = []
    for i in range(N_CORES):
        fs = slice(i * F_LOC, (i + 1) * F_LOC)
        in_maps.append({
            "xT": xT,
            "wqT": _mm_np(np.asarray(wq)[fs, :].T),
            "wkT": _mm_np(np.asarray(wk)[fs, :].T),
            "wvT": _mm_np(np.asarray(wv)[fs, :].T),
            "woT": _mm_np(np.asarray(wo)[:, fs].T),
            "bq2": np.ascontiguousarray(
                np.asarray(bq)[fs].reshape(H_LOC, HD).T),
            "bk2": np.ascontiguousarray(
                np.asarray(bk)[fs].reshape(H_LOC, HD).T),
            "bvr": np.ascontiguousarray(np.broadcast_to(
                np.asarray(bv, dtype=np.float32)[fs][None, :], (128, F_LOC))),
            "ones": _mm_np(np.ones((128, 128), dtype=np.float32)),
        })
    return in_maps


def kernel(x, wq, bq, wk, bk, wv, bv, wo, bo):
    from concourse.bass_utils import run_bass_kernel_spmd

    nc = _get_nc()
    in_maps = make_in_maps(x, wq, bq, wk, bk, wv, bv, wo)
    res = run_bass_kernel_spmd(nc, in_maps, core_ids=list(range(N_CORES)),
                               trace=False)
    y = np.zeros((B * S, D), dtype=np.float32)
    for i in range(N_CORES):
        y += res.results[i]["y"]
    y += np.asarray(bo, dtype=np.float32)[None, :]
    return y.reshape(B, S, D)



# revision 9
# speedup vs baseline: 1.2133x; 1.2133x over previous
"""Causal multi-head attention on 8 Trainium2 NeuronCores.

Sharding: Megatron-style tensor parallelism over heads. Each of the 8
cores computes 2 of the 16 heads end-to-end:
  - column-parallel Q/K/V projections (each core owns a 256-feature slice
    of wq/wk/wv),
  - per-head causal attention entirely on-core,
  - row-parallel output projection producing a partial [B*S, D] output.
The 8 partials are summed on the host (the "all-reduce") and bo added.

Numerics / engine layout:
  - Q/K/V projections run on the PE in fp8e4 DoubleRow perf mode (256-deep
    contraction at 0.5 cycles/col, 4x bf16 FLOP rate) with 3-term error
    compensation: x8*w8 + x8*wr + xr*w8 where x8/xr and w8/wr are fp8
    value/residual pairs.  Residual error ~0.15%, at 75% of the bf16 cost.
  - v's bias is folded into the host-side output bias (bo += wo @ bv); v is
    stored as an fp8 value/residual pair (v8, vr) for the PV matmul.
  - Scores are computed transposed ([k, q]) in bf16.  For query chunks
    c>=1 the exp'd probabilities are written as fp8e4 with a per-chunk
    shift C_c (exp(s/sqrt(hd) - C_c); numerator and denominator share the
    shift so it cancels), and PV / rowsum run as fp8 DoubleRow matmuls
    contracting 256 keys per instruction (PV has 2 terms: e*v8 + e*vr).
    Chunk 0 (queries 0..511, where short rows make fp8 exp underflow)
    keeps the bf16 path; it is only ~10% of the attention work.
  - The output projection stays bf16; partial y is written back as bf16
    and the 8 partials are summed on the host in fp32.
  - Causality at block granularity: upper-triangle blocks are never
    computed; diagonal blocks get a static additive mask; the DoubleRow
    key-block pairing makes the odd diagonal blocks' sub-diagonal strips
    part of the moving operand, so those e strips are memset to zero.
"""

import math

import numpy as np

B = 2
S = 2048
D = 2048
H = 16
HD = 128  # head dim
N_CORES = 8
H_LOC = H // N_CORES       # 2 heads per core
F_LOC = H_LOC * HD         # 256 local features per core
KT = D // 128              # 16 contraction tiles
KP = KT // 2               # 8 DoubleRow contraction pairs
CHUNK = 512                # token chunk (matmul moving dim)
NCH = S // CHUNK           # 4 chunks per batch
TT = S // 128              # 16 token tiles per batch

# per-chunk exp shift for the fp8 probability path (c=0 unused: bf16 path)
C_SHIFT = [0.0, 4.5, 4.5, 5.0]
# host-side weight scaling: keeps the fp8 weight residuals out of the
# subnormal range.  q/k copies divide it back out; for v the scale rides
# into PV and cancels against a rowsum computed with ones*W_SCALE.
W_SCALE = 32.0

_CACHE = {}


def _build(reps=None):
    import concourse.mybir as mybir
    import concourse.tile as tile
    from concourse import bacc

    F32 = mybir.dt.float32
    BF16 = mybir.dt.bfloat16
    FP8E4 = mybir.dt.float8e4
    DRM = mybir.MatmulPerfMode.DoubleRow
    ADD = mybir.AluOpType.add
    SUB = mybir.AluOpType.subtract
    MULT = mybir.AluOpType.mult
    EXP = mybir.ActivationFunctionType.Exp
    INV_SQRT_HD = 1.0 / math.sqrt(HD)

    nc = bacc.Bacc("TRN2", target_bir_lowering=False, debug=False,
                   num_devices=N_CORES)

    x8T_d = nc.dram_tensor("x8T", [D, B * S], FP8E4, kind="ExternalInput")
    xrT_d = nc.dram_tensor("xrT", [D, B * S], FP8E4, kind="ExternalInput")
    w8_d = {}
    wr_d = {}
    for nm in ("wq", "wk", "wv"):
        w8_d[nm] = nc.dram_tensor(nm + "8T", [D, F_LOC], FP8E4,
                                  kind="ExternalInput")
        wr_d[nm] = nc.dram_tensor(nm + "rT", [D, F_LOC], FP8E4,
                                  kind="ExternalInput")
    woT_d = nc.dram_tensor("woT", [F_LOC, D], BF16, kind="ExternalInput")
    bq_d = nc.dram_tensor("bq2", [HD, H_LOC], F32, kind="ExternalInput")
    bk_d = nc.dram_tensor("bk2", [HD, H_LOC], F32, kind="ExternalInput")
    ones_d = nc.dram_tensor("ones", [128, 2 * 128], FP8E4,
                            kind="ExternalInput")
    y_d = nc.dram_tensor("y", [B * S, D], BF16, kind="ExternalOutput")

    with tile.TileContext(nc) as tc:
        cpool = tc.alloc_tile_pool(name="const", bufs=1)
        wpool = tc.alloc_tile_pool(name="w", bufs=1)
        xkpool = tc.alloc_tile_pool(name="xk", bufs=6)
        actpool = tc.alloc_tile_pool(name="act", bufs=6)
        ypool = tc.alloc_tile_pool(name="y", bufs=4)
        ripool = tc.alloc_tile_pool(name="ri", bufs=2)
        psq = tc.alloc_tile_pool(name="ps", bufs=5, space="PSUM")
        psa = psr = psq
        pso = tc.alloc_tile_pool(name="pso", bufs=3, space="PSUM")
        qtag = "ps"

        def kview(t, width=CHUNK):
            """[128, KT*width] tile -> [128, KT, width] k-tile view."""
            return t[:].rearrange("p (k f) -> p k f", k=KT)

        # --- loads; DMA queue order is deliberate (wq + first x chunks
        # first so the projection matmuls start a few us in) ---
        def load_w(nm, dram):
            w_t = wpool.tile([128, KT * F_LOC], FP8E4, tag=nm, name=nm)
            nc.sync.dma_start(
                w_t[:].rearrange("p (k f) -> p k f", k=KT),
                dram.ap().rearrange("(k p) f -> p k f", p=128),
            )
            return w_t

        def load_x(b, c, split=1):
            """Load one 512-token chunk of x8 and xr."""
            xs = []
            for src in (x8T_d, xrT_d):
                x_t = xkpool.tile([128, KT * CHUNK], FP8E4, tag="xk",
                                  name="x_t")
                col0 = b * S + c * CHUNK
                kstep = KT // split
                for s in range(split):
                    k0 = s * kstep
                    nc.sync.dma_start(
                        x_t[:, k0 * CHUNK:(k0 + kstep) * CHUNK]
                            .rearrange("p (k f) -> p k f", k=kstep),
                        src.ap()[k0 * 128:(k0 + kstep) * 128,
                                 col0:col0 + CHUNK]
                            .rearrange("(k p) f -> p k f", p=128),
                    )
                xs.append(x_t)
            return xs

        w_ts = {"wq": load_w("wq", w8_d["wq"]),
                "wqr": load_w("wqr", wr_d["wq"])}
        x_first = load_x(0, 0, split=4)
        w_ts["wk"] = load_w("wk", w8_d["wk"])
        w_ts["wkr"] = load_w("wkr", wr_d["wk"])
        w_ts["wv"] = load_w("wv", w8_d["wv"])
        w_ts["wvr"] = load_w("wvr", wr_d["wv"])
        x_second = load_x(0, 1, split=2)
        bq_t = cpool.tile([HD, H_LOC], F32, tag="bq")
        bk_t = cpool.tile([HD, H_LOC], F32, tag="bk")
        nc.sync.dma_start(bq_t[:], bq_d.ap())
        nc.sync.dma_start(bk_t[:], bk_d.ap())
        ones8 = cpool.tile([128, 2 * 128], FP8E4, tag="ones8")
        nc.sync.dma_start(ones8[:], ones_d.ap())
        # bf16 "ones" for the chunk-0 rowsum (scaled to match the scaled v)
        ones16 = cpool.tile([128, 128], BF16, tag="ones16")
        nc.gpsimd.memset(ones16[:], W_SCALE)
        # per-chunk exp-shift bias tiles (activation bias must be an AP)
        shift_t = {}
        for c in range(1, NCH):
            sh = cpool.tile([128, 1], F32, tag=f"shift{c}", name="sh")
            nc.gpsimd.memset(sh[:], -C_SHIFT[c])
            shift_t[c] = sh
        # warm the ACT Exp table during the QKV phase so the table load
        # doesn't land on the first attention chunk
        warm_t = cpool.tile([128, 1], F32, tag="warm")
        nc.scalar.activation(warm_t[:], bq_t[:, 0:1], EXP,
                             bias=0.0, scale=1.0)
        maskT = cpool.tile([128, 128], F32, tag="maskT")
        nc.gpsimd.memset(maskT[:], 0.0)
        # transposed causal mask: keep (0) where k_part <= q_free else -1e9
        nc.gpsimd.affine_select(
            out=maskT[:], in_=maskT[:],
            compare_op=mybir.AluOpType.is_ge,
            fill=-1e9, base=0, pattern=[[1, 128]], channel_multiplier=-1,
        )

        def qkv_chunk(x_ts, c, acts):
            """Q/K/V projections for one 512-token chunk (fp8 DR, 3-term)."""
            x8_t, xr_t = x_ts
            qT_t, kT_t, v8_t, vr_t, _ = acts
            for nm, dst, bias in (("wq", qT_t, bq_t), ("wk", kT_t, bk_t)):
                w8v = kview(w_ts[nm], F_LOC)
                wrv = kview(w_ts[nm + "r"], F_LOC)
                for h in range(H_LOC):
                    hs = slice(h * HD, (h + 1) * HD)
                    q_ps = psq.tile([128, CHUNK], F32, tag=qtag, name="q_ps")
                    terms = [(w8v, x8_t), (wrv, x8_t), (w8v, xr_t)]
                    n = 3 * KP
                    i = 0
                    for wv_, xv_ in terms:
                        xk = kview(xv_)
                        for k2 in range(KP):
                            nc.tensor.matmul(
                                q_ps[:],
                                wv_[:, 2 * k2:2 * k2 + 2, hs],
                                xk[:, 2 * k2:2 * k2 + 2, :],
                                start=(i == 0), stop=(i == n - 1),
                                perf_mode=DRM,
                            )
                            i += 1
                    # psum holds 32*(x@w); fold the 1/32 into the bias copy
                    nc.vector.tensor_scalar(
                        dst[:, h * S + c * CHUNK: h * S + (c + 1) * CHUNK],
                        q_ps[:], 1.0 / W_SCALE, bias[:, h:h + 1],
                        op0=MULT, op1=ADD)
            w8v = kview(w_ts["wv"], F_LOC)
            wrv = kview(w_ts["wvr"], F_LOC)
            for t4 in range(CHUNK // 128):
                tt = c * (CHUNK // 128) + t4
                ts4 = slice(t4 * 128, (t4 + 1) * 128)
                v_ps = psq.tile([128, CHUNK], F32, tag=qtag, name="v_ps")
                terms = [(x8_t, w8v), (x8_t, wrv), (xr_t, w8v)]
                n = 3 * KP
                i = 0
                for xv_, wv_ in terms:
                    xk = kview(xv_)
                    for k2 in range(KP):
                        nc.tensor.matmul(
                            v_ps[:, 0:F_LOC],
                            xk[:, 2 * k2:2 * k2 + 2, ts4],
                            wv_[:, 2 * k2:2 * k2 + 2, :],
                            start=(i == 0), stop=(i == n - 1),
                            perf_mode=DRM,
                        )
                        i += 1
                # v value/residual fp8 pair (bias folded into host bo)
                nc.vector.tensor_copy(
                    v8_t[:, tt * F_LOC:(tt + 1) * F_LOC], v_ps[:, 0:F_LOC])
                nc.vector.tensor_tensor(
                    vr_t[:, tt * F_LOC:(tt + 1) * F_LOC],
                    v_ps[:, 0:F_LOC],
                    v8_t[:, tt * F_LOC:(tt + 1) * F_LOC], SUB)

        def attn_chunk0(acts):
            """Chunk 0: bf16 probabilities (short rows underflow fp8)."""
            qT_t, kT_t, v8_t, vr_t, attnT_t = acts
            nki = 4
            for h in range(H_LOC):
            # e in bf16; v = v8 + vr summed on the fly is not possible in
            # a bf16 matmul, so PV runs two matmul accumulation terms.
                e_t = xkpool.tile([128, nki * CHUNK], BF16, tag="e0",
                                  bufs=2, name="e_t")
                q0 = h * S
                for ki in range(nki):
                    trim = 128 * ki
                    ncol = CHUNK - trim
                    s_ps = psa.tile([128, CHUNK], F32, tag=qtag, name="s_ps")
                    nc.tensor.matmul(
                        s_ps[:, 0:ncol],
                        kT_t[:, h * S + ki * 128: h * S + (ki + 1) * 128],
                        qT_t[:, q0 + trim: q0 + CHUNK],
                        start=True, stop=True,
                    )
                    nc.vector.tensor_tensor(
                        s_ps[:, 0:128], s_ps[:, 0:128], maskT[:], ADD)
                    nc.scalar.activation(
                        e_t[:, ki * CHUNK + trim:(ki + 1) * CHUNK],
                        s_ps[:, 0:ncol], EXP, bias=0.0, scale=INV_SQRT_HD)
                at_ps = psr.tile([128, CHUNK], F32, tag=qtag, name="at_ps")
                rs_ps = psr.tile([128, CHUNK], F32, tag=qtag, name="rs_ps")
                for term in range(2):
                    v_t = (v8_t, vr_t)[term]
                    for ki in range(nki):
                        trim = 128 * ki
                        nc.tensor.matmul(
                            at_ps[:, trim:CHUNK],
                            v_t[:, ki * F_LOC + h * HD:
                                ki * F_LOC + (h + 1) * HD],
                            e_t[:, ki * CHUNK + trim:(ki + 1) * CHUNK],
                            start=(term == 0 and ki == 0),
                            stop=(term == 1 and ki == nki - 1),
                        )
                for ki in range(nki):
                    trim = 128 * ki
                    nc.tensor.matmul(
                        rs_ps[:, trim:CHUNK],
                        ones16[:],
                        e_t[:, ki * CHUNK + trim:(ki + 1) * CHUNK],
                        start=(ki == 0), stop=(ki == nki - 1),
                    )
                ri_t = ripool.tile([128, CHUNK], F32, tag="ri", name="ri_t")
                nc.vector.reciprocal(ri_t[:], rs_ps[:])
                nc.vector.tensor_tensor(
                    attnT_t[:, q0: q0 + CHUNK],
                    at_ps[:], ri_t[:], MULT)

        def attn_chunk(c, acts):
            """Causal attention for one 512-query chunk (fp8 DR), c>=1."""
            qT_t, kT_t, v8_t, vr_t, attnT_t = acts
            nki = 4 * c + 4
            npair = nki // 2
            shift = shift_t[c]
            for h in range(H_LOC):
                e_t = xkpool.tile([128, KT * CHUNK], FP8E4, tag="xk",
                                  name="e_t")
                ek = kview(e_t)
                q0 = h * S + c * CHUNK
                # scoresT blocks + exp (transposed layout: [k, q]), bf16 PE
                for ki in range(nki):
                    r = ki - 4 * c
                    trim = 128 * r if r > 0 else 0
                    ncol = CHUNK - trim
                    s_ps = psa.tile([128, CHUNK], F32, tag=qtag, name="s_ps")
                    nc.tensor.matmul(
                        s_ps[:, 0:ncol],
                        kT_t[:, h * S + ki * 128: h * S + (ki + 1) * 128],
                        qT_t[:, q0 + trim: q0 + CHUNK],
                        start=True, stop=True,
                    )
                    if ki >= 4 * c:  # diagonal 128x128 needs the mask
                        nc.vector.tensor_tensor(
                            s_ps[:, 0:128], s_ps[:, 0:128], maskT[:], ADD)
                    nc.scalar.activation(
                        e_t[:, ki * CHUNK + trim:(ki + 1) * CHUNK],
                        s_ps[:, 0:ncol], EXP, bias=shift[:],
                        scale=INV_SQRT_HD)
                # zero the sub-diagonal strips of the odd diagonal blocks
                # (they sit inside the DoubleRow pair's shared column span)
                nc.gpsimd.memset(
                    e_t[:, (4 * c + 1) * CHUNK:(4 * c + 1) * CHUNK + 128],
                    0.0)
                nc.gpsimd.memset(
                    e_t[:, (4 * c + 3) * CHUNK + 256:
                        (4 * c + 3) * CHUNK + 384],
                    0.0)
                # PV (2 terms: v8, vr) and rowsum over key-block pairs
                v8k = v8_t[:].rearrange("p (k f) -> p k f", k=TT)
                vrk = vr_t[:].rearrange("p (k f) -> p k f", k=TT)
                hs = slice(h * HD, (h + 1) * HD)
                at_ps = psr.tile([128, CHUNK], F32, tag=qtag, name="at_ps")
                rs_ps = psr.tile([128, CHUNK], F32, tag=qtag, name="rs_ps")
                n = 2 * npair
                i = 0
                for term in range(2):
                    vk = (v8k, vrk)[term]
                    for kp in range(npair):
                        ptrim = 256 if kp == npair - 1 else 0
                        nc.tensor.matmul(
                            at_ps[:, ptrim:CHUNK],
                            vk[:, 2 * kp:2 * kp + 2, hs],
                            ek[:, 2 * kp:2 * kp + 2, ptrim:CHUNK],
                            start=(i == 0), stop=(i == n - 1),
                            perf_mode=DRM,
                        )
                        i += 1
                for kp in range(npair):
                    ptrim = 256 if kp == npair - 1 else 0
                    nc.tensor.matmul(
                        rs_ps[:, ptrim:CHUNK],
                        ones8[:].rearrange("p (k f) -> p k f", k=2),
                        ek[:, 2 * kp:2 * kp + 2, ptrim:CHUNK],
                        start=(kp == 0), stop=(kp == npair - 1),
                        perf_mode=DRM,
                    )
                ri_t = ripool.tile([128, CHUNK], F32, tag="ri", name="ri_t")
                nc.vector.reciprocal(ri_t[:], rs_ps[:])
                nc.vector.tensor_tensor(
                    attnT_t[:, q0: q0 + CHUNK],
                    at_ps[:], ri_t[:], MULT)

        def outproj_chunk(b, c, acts, wo_t):
            """Output projection + y writeback for one chunk's tokens."""
            attnT_t = acts[4]
            for t4 in range(CHUNK // 128):
                tt = c * (CHUNK // 128) + t4
                for oc in range(D // CHUNK):
                    o_ps = pso.tile([128, CHUNK], F32, tag="pso",
                                    name="o_ps")
                    for h in range(H_LOC):
                        nc.tensor.matmul(
                            o_ps[:],
                            attnT_t[:, h * S + tt * 128:
                                    h * S + (tt + 1) * 128],
                            wo_t[:, h * D + oc * CHUNK:
                                 h * D + (oc + 1) * CHUNK],
                            start=(h == 0), stop=(h == H_LOC - 1),
                        )
                    y_t = ypool.tile([128, CHUNK], BF16, tag="y", name="y_t")
                    nc.vector.tensor_copy(y_t[:], o_ps[:])
                    row0 = b * S + tt * 128
                    nc.sync.dma_start(
                        y_d.ap()[row0:row0 + 128,
                                 oc * CHUNK:(oc + 1) * CHUNK], y_t[:])

        def new_acts():
            qT_t = actpool.tile([128, H_LOC * S], BF16, tag="act",
                                name="qT_t")
            kT_t = actpool.tile([128, H_LOC * S], BF16, tag="act",
                                name="kT_t")
            v8_t = actpool.tile([128, TT * F_LOC], FP8E4, tag="actv",
                                bufs=2, name="v8_t")
            vr_t = actpool.tile([128, TT * F_LOC], FP8E4, tag="actvr",
                                bufs=2, name="vr_t")
            attnT_t = actpool.tile([128, H_LOC * S], BF16, tag="act",
                                   name="attnT_t")
            return (qT_t, kT_t, v8_t, vr_t, attnT_t)

        def load_wo():
            # woT [F_LOC, D] -> [128, H_LOC*D]; deferred load so the DMA
            # queue prioritizes x chunks during warmup
            wo_t = wpool.tile([128, H_LOC * D], BF16, tag="wo", name="wo_t")
            nc.sync.dma_start(
                wo_t[:].rearrange("p (h f) -> p h f", h=H_LOC),
                woT_d.ap().rearrange("(h p) f -> p h f", p=128),
            )
            return wo_t

        def emit_body(first_iter=True):
            wo_t = None
            for b in range(B):
                acts = new_acts()
                for c in range(NCH):
                    if first_iter and b == 0 and c == 0:
                        x_ts = x_first
                    elif first_iter and b == 0 and c == 1:
                        x_ts = x_second
                    else:
                        x_ts = load_x(b, c,
                                      split=(4 if (b == 0 and c == 0)
                                             else 1))
                    if wo_t is None and c == NCH - 1:
                        wo_t = load_wo()
                    qkv_chunk(x_ts, c, acts)
                for c in range(NCH):
                    if c == 0:
                        attn_chunk0(acts)
                    else:
                        attn_chunk(c, acts)
                    outproj_chunk(b, c, acts, wo_t)

        if reps is None:
            emit_body()
        else:
            with tc.For_i(0, reps, 1):
                emit_body(first_iter=False)

        pools = [pso, psq, ripool, ypool, actpool, xkpool,
                 wpool, cpool]
        seen = set()
        for p in pools:
            if id(p) not in seen:
                seen.add(id(p))
                p.release()

    nc.compile()
    return nc


def _get_nc(reps=None):
    key = ("nc", reps)
    if key not in _CACHE:
        _CACHE[key] = _build(reps)
    return _CACHE[key]


def _fp8(a):
    import ml_dtypes
    return np.ascontiguousarray(a).astype(ml_dtypes.float8_e4m3)


def _bf16(a):
    import ml_dtypes
    return np.ascontiguousarray(a).astype(ml_dtypes.bfloat16)


def make_in_maps(x, wq, bq, wk, bk, wv, bv, wo):
    x = np.asarray(x, dtype=np.float32)
    xT = np.ascontiguousarray(x.reshape(B * S, D).T)  # [D, B*S]
    x8T = _fp8(xT)
    xrT = _fp8(xT - x8T.astype(np.float32))

    in_maps = []
    for i in range(N_CORES):
        fs = slice(i * F_LOC, (i + 1) * F_LOC)
        m = {
            "x8T": x8T,
            "xrT": xrT,
            "woT": _bf16(np.asarray(wo)[:, fs].T),
            "bq2": np.ascontiguousarray(
                np.asarray(bq)[fs].reshape(H_LOC, HD).T.astype(np.float32)),
            "bk2": np.ascontiguousarray(
                np.asarray(bk)[fs].reshape(H_LOC, HD).T.astype(np.float32)),
            "ones": _fp8(np.full((128, 256), W_SCALE, dtype=np.float32)),
        }
        for nm, w in (("wq", wq), ("wk", wk), ("wv", wv)):
            wT = W_SCALE * np.asarray(w, dtype=np.float32)[fs, :].T
            w8 = _fp8(wT)
            m[nm + "8T"] = w8
            m[nm + "rT"] = _fp8(wT - w8.astype(np.float32))
        in_maps.append(m)
    return in_maps


def kernel(x, wq, bq, wk, bk, wv, bv, wo, bo):
    from concourse.bass_utils import run_bass_kernel_spmd

    nc = _get_nc()
    in_maps = make_in_maps(x, wq, bq, wk, bk, wv, bv, wo)
    res = run_bass_kernel_spmd(nc, in_maps, core_ids=list(range(N_CORES)),
                               trace=False)
    y = np.zeros((B * S, D), dtype=np.float32)
    for i in range(N_CORES):
        y += res.results[i]["y"].astype(np.float32)
    # v bias folded through the output projection, plus bo
    y += (np.asarray(wo, dtype=np.float32) @ np.asarray(bv, np.float32)
          + np.asarray(bo, np.float32))[None, :]
    return y.reshape(B, S, D)


# revision 29
# speedup vs baseline: 1.2767x; 1.0523x over previous
"""Causal multi-head attention on 8 Trainium2 NeuronCores.

Sharding: Megatron-style tensor parallelism over heads. Each of the 8
cores computes 2 of the 16 heads end-to-end:
  - column-parallel Q/K/V projections (each core owns a 256-feature slice
    of wq/wk/wv),
  - per-head causal attention entirely on-core,
  - row-parallel output projection producing a partial [B*S, D] output.
The 8 partials are summed on the host (the "all-reduce") and bo added.

Numerics / engine layout:
  - Q/K/V projections run on the PE in fp8e4 DoubleRow perf mode (256-deep
    contraction at 0.5 cycles/col, 4x bf16 FLOP rate) with 3-term error
    compensation: x8*w8 + x8*wr + xr*w8 where x8/xr and w8/wr are fp8
    value/residual pairs.  Residual error ~0.15%, at 75% of the bf16 cost.
  - v's bias is folded into the host-side output bias (bo += wo @ bv); v is
    stored as an fp8 value/residual pair (v8, vr) for the PV matmul.
  - Scores are computed transposed ([k, q]) in bf16.  For query chunks
    c>=1 the exp'd probabilities are written as fp8e4 with a per-chunk
    shift C_c (exp(s/sqrt(hd) - C_c); numerator and denominator share the
    shift so it cancels), and PV / rowsum run as fp8 DoubleRow matmuls
    contracting 256 keys per instruction (PV has 2 terms: e*v8 + e*vr).
    Chunk 0 (queries 0..511, where short rows make fp8 exp underflow)
    keeps the bf16 path; it is only ~10% of the attention work.
  - The output projection stays bf16; partial y is written back as bf16
    and the 8 partials are summed on the host in fp32.
  - Causality at block granularity: upper-triangle blocks are never
    computed; diagonal blocks get a static additive mask; the DoubleRow
    key-block pairing makes the odd diagonal blocks' sub-diagonal strips
    part of the moving operand, so those e strips are memset to zero.
"""

import math

import numpy as np

B = 2
S = 2048
D = 2048
H = 16
HD = 128  # head dim
N_CORES = 8
H_LOC = H // N_CORES       # 2 heads per core
F_LOC = H_LOC * HD         # 256 local features per core
KT = D // 128              # 16 contraction tiles
KP = KT // 2               # 8 DoubleRow contraction pairs
CHUNK = 512                # token chunk (matmul moving dim)
NCH = S // CHUNK           # 4 chunks per batch
TT = S // 128              # 16 token tiles per batch

# per-chunk exp shift for the fp8 probability path (c=0 unused: bf16 path)
C_SHIFT = [0.0, 4.5, 4.5, 5.0]
# host-side weight scaling: keeps the fp8 weight residuals out of the
# subnormal range.  q/k copies divide it back out; for v the scale rides
# into PV and cancels against a rowsum computed with ones*W_SCALE.
W_SCALE = 32.0

_CACHE = {}


def _build(reps=None):
    import concourse.mybir as mybir
    import concourse.tile as tile
    from concourse import bacc

    F32 = mybir.dt.float32
    BF16 = mybir.dt.bfloat16
    FP8E4 = mybir.dt.float8e4
    DRM = mybir.MatmulPerfMode.DoubleRow
    ADD = mybir.AluOpType.add
    SUB = mybir.AluOpType.subtract
    MULT = mybir.AluOpType.mult
    EXP = mybir.ActivationFunctionType.Exp
    INV_SQRT_HD = 1.0 / math.sqrt(HD)

    nc = bacc.Bacc("TRN2", target_bir_lowering=False, debug=False,
                   num_devices=N_CORES)

    xiT_d = nc.dram_tensor("xiT", [2 * D, B * S], FP8E4,
                           kind="ExternalInput")
    w8_d = {}
    wr_d = {}
    for nm in ("wq", "wk", "wv"):
        w8_d[nm] = nc.dram_tensor(nm + "8T", [D, F_LOC], FP8E4,
                                  kind="ExternalInput")
        wr_d[nm] = nc.dram_tensor(nm + "rT", [D, F_LOC], FP8E4,
                                  kind="ExternalInput")
    wo8_d = nc.dram_tensor("wo8T", [F_LOC, D], FP8E4, kind="ExternalInput")
    wor_d = nc.dram_tensor("worT", [F_LOC, D], FP8E4, kind="ExternalInput")
    bq_d = nc.dram_tensor("bq2", [HD, H_LOC], F32, kind="ExternalInput")
    bk_d = nc.dram_tensor("bk2", [HD, H_LOC], F32, kind="ExternalInput")
    ones_d = nc.dram_tensor("ones", [128, 2 * 128], FP8E4,
                            kind="ExternalInput")
    id_d = nc.dram_tensor("id128", [128, 128], BF16, kind="ExternalInput")
    y_d = nc.dram_tensor("y", [B * S, D], BF16, kind="ExternalOutput")

    with tile.TileContext(nc) as tc:
        cpool = tc.alloc_tile_pool(name="const", bufs=1)
        wpool = tc.alloc_tile_pool(name="w", bufs=1)
        xkpool = tc.alloc_tile_pool(name="xk", bufs=3)
        actpool = tc.alloc_tile_pool(name="act", bufs=6)
        ypool = tc.alloc_tile_pool(name="y", bufs=4)
        ripool = tc.alloc_tile_pool(name="ri", bufs=2)
        psq = tc.alloc_tile_pool(name="ps", bufs=5, space="PSUM")
        psa = psr = psq
        pso = tc.alloc_tile_pool(name="pso", bufs=3, space="PSUM")
        qtag = "ps"

        def kview(t, width=CHUNK):
            """[128, KT*width] tile -> [128, KT, width] k-tile view."""
            return t[:].rearrange("p (k f) -> p k f", k=KT)

        # --- loads; DMA queue order is deliberate (wq + first x chunks
        # first so the projection matmuls start a few us in) ---
        def load_w(nm, dram, eng=None):
            w_t = wpool.tile([128, KT * F_LOC], FP8E4, tag=nm, name=nm)
            (eng or nc.sync).dma_start(
                w_t[:].rearrange("p (k f) -> p k f", k=KT),
                dram.ap().rearrange("(k p) f -> p k f", p=128),
            )
            return w_t

        def load_x(b, c, split=1):
            """Load one 512-token chunk of packed x (x8+xr interleaved by
            k-tile) as a single transfer stream so arrival order matches
            the projection matmuls' consumption order."""
            x_t = xkpool.tile([128, KT * 2 * CHUNK], FP8E4, tag="xk",
                              name="x_t")
            col0 = b * S + c * CHUNK
            kstep = KT // split
            for s in range(split):
                k0 = s * kstep
                nc.sync.dma_start(
                    x_t[:, k0 * 2 * CHUNK:(k0 + kstep) * 2 * CHUNK]
                        .rearrange("p (k f) -> p k f", k=2 * kstep),
                    xiT_d.ap()[k0 * 256:(k0 + kstep) * 256,
                               col0:col0 + CHUNK]
                        .rearrange("(k p) f -> p k f", p=128),
                )
            return x_t

        # startup order: wq8 -> x8(c0) -> wqr -> xr(c0) so the first
        # projection matmuls (term x8@w8) start as early as possible
        w_ts = {"wq": load_w("wq", w8_d["wq"])}
        w_ts["wqr"] = load_w("wqr", wr_d["wq"])
        x_first = load_x(0, 0, split=4)
        w_ts["wk"] = load_w("wk", w8_d["wk"])
        w_ts["wkr"] = load_w("wkr", wr_d["wk"])
        w_ts["wv"] = load_w("wv", w8_d["wv"])
        w_ts["wvr"] = load_w("wvr", wr_d["wv"])
        x_second = load_x(0, 1, split=2)
        bq_t = cpool.tile([HD, H_LOC], F32, tag="bq")
        bk_t = cpool.tile([HD, H_LOC], F32, tag="bk")
        nc.sync.dma_start(bq_t[:], bq_d.ap())
        nc.sync.dma_start(bk_t[:], bk_d.ap())
        ones8 = cpool.tile([128, 2 * 128], FP8E4, tag="ones8")
        nc.sync.dma_start(ones8[:], ones_d.ap())
        id128 = cpool.tile([128, 128], BF16, tag="id128")
        nc.sync.dma_start(id128[:], id_d.ap())
        # bf16 "ones" for the chunk-0 rowsum (scaled to match the scaled v)
        ones16 = cpool.tile([128, 128], BF16, tag="ones16")
        nc.gpsimd.memset(ones16[:], W_SCALE)
        # per-chunk exp-shift bias tiles (activation bias must be an AP)
        shift_t = {}
        for c in range(1, NCH):
            sh = cpool.tile([128, 1], F32, tag=f"shift{c}", name="sh")
            nc.gpsimd.memset(sh[:], -C_SHIFT[c])
            shift_t[c] = sh
        # warm the ACT Exp table during the QKV phase so the table load
        # doesn't land on the first attention chunk
        warm_t = cpool.tile([128, 1], F32, tag="warm")
        nc.scalar.activation(warm_t[:], bq_t[:, 0:1], EXP,
                             bias=0.0, scale=1.0)
        maskT = cpool.tile([128, 128], BF16, tag="maskT")
        nc.gpsimd.memset(maskT[:], 0.0)
        # transposed causal mask: keep (0) where k_part <= q_free else -1e9;
        # bf16 so it can ride into the scores PSUM as an extra accumulating
        # matmul (identity stationary) instead of a DVE pass
        nc.gpsimd.affine_select(
            out=maskT[:], in_=maskT[:],
            compare_op=mybir.AluOpType.is_ge,
            fill=-1e9, base=0, pattern=[[1, 128]], channel_multiplier=-1,
        )

        def qkv_chunk(x_t, c, acts):
            """Q/K/V projections for one 512-token chunk (fp8 DR, 3-term).

            x_t is the packed chunk: free layout (k, two, f) with two=0
            holding x8 and two=1 holding the xr residual."""
            xk = x_t[:].rearrange("p (k two f) -> p k two f", k=KT, two=2)
            qT_t, kT_t, v8_t, vr_t, _ = acts
            for nm, dst, bias in (("wq", qT_t, bq_t), ("wk", kT_t, bk_t)):
                w8v = kview(w_ts[nm], F_LOC)
                wrv = kview(w_ts[nm + "r"], F_LOC)
                for h in range(H_LOC):
                    hs = slice(h * HD, (h + 1) * HD)
                    q_ps = psq.tile([128, CHUNK], F32, tag=qtag, name="q_ps")
                    terms = [(w8v, 0), (wrv, 0), (w8v, 1)]
                    n = 3 * KP
                    i = 0
                    for wv_, sel in terms:
                        for k2 in range(KP):
                            nc.tensor.matmul(
                                q_ps[:],
                                wv_[:, 2 * k2:2 * k2 + 2, hs],
                                xk[:, 2 * k2:2 * k2 + 2, sel, :],
                                start=(i == 0), stop=(i == n - 1),
                                perf_mode=DRM,
                            )
                            i += 1
                    # psum holds 32*(x@w); fold the 1/32 into the bias copy
                    nc.vector.tensor_scalar(
                        dst[:, h * S + c * CHUNK: h * S + (c + 1) * CHUNK],
                        q_ps[:], 1.0 / W_SCALE, bias[:, h:h + 1],
                        op0=MULT, op1=ADD)
            w8v = kview(w_ts["wv"], F_LOC)
            wrv = kview(w_ts["wvr"], F_LOC)
            for t4 in range(CHUNK // 128):
                tt = c * (CHUNK // 128) + t4
                ts4 = slice(t4 * 128, (t4 + 1) * 128)
                v_ps = psq.tile([128, CHUNK], F32, tag=qtag, name="v_ps")
                terms = [(0, w8v), (0, wrv), (1, w8v)]
                n = 3 * KP
                i = 0
                for sel, wv_ in terms:
                    for k2 in range(KP):
                        nc.tensor.matmul(
                            v_ps[:, 0:F_LOC],
                            xk[:, 2 * k2:2 * k2 + 2, sel, ts4],
                            wv_[:, 2 * k2:2 * k2 + 2, :],
                            start=(i == 0), stop=(i == n - 1),
                            perf_mode=DRM,
                        )
                        i += 1
                # v value/residual fp8 pair (bias folded into host bo)
                nc.vector.tensor_copy(
                    v8_t[:, tt * F_LOC:(tt + 1) * F_LOC], v_ps[:, 0:F_LOC])
                nc.vector.tensor_tensor(
                    vr_t[:, tt * F_LOC:(tt + 1) * F_LOC],
                    v_ps[:, 0:F_LOC],
                    v8_t[:, tt * F_LOC:(tt + 1) * F_LOC], SUB)

        def attn0_scores(acts):
            """Chunk 0 scores+exp: bf16 probabilities (short rows
            underflow fp8)."""
            qT_t, kT_t, v8_t, vr_t, attnT_t = acts
            nki = 4
            e_ts = []
            for h in range(H_LOC):
                # e in bf16; v = v8 + vr summed on the fly is not possible
                # in a bf16 matmul, so PV runs two matmul accumulation terms.
                e_t = xkpool.tile([128, nki * CHUNK], BF16, tag="e0",
                                  bufs=2, name="e_t")
                e_ts.append(e_t)
                q0 = h * S
                for ki in range(nki):
                    trim = 128 * ki
                    ncol = CHUNK - trim
                    s_ps = psa.tile([128, CHUNK], F32, tag=qtag, name="s_ps")
                    nc.tensor.matmul(
                        s_ps[:, 0:ncol],
                        kT_t[:, h * S + ki * 128: h * S + (ki + 1) * 128],
                        qT_t[:, q0 + trim: q0 + CHUNK],
                        start=True, stop=False,
                    )
                    nc.tensor.matmul(
                        s_ps[:, 0:128], id128[:], maskT[:],
                        start=False, stop=True,
                    )
                    nc.scalar.activation(
                        e_t[:, ki * CHUNK + trim:(ki + 1) * CHUNK],
                        s_ps[:, 0:ncol], EXP, bias=0.0, scale=INV_SQRT_HD)
            return e_ts

        def attn0_pv(acts, e_ts):
            qT_t, kT_t, v8_t, vr_t, attnT_t = acts
            nki = 4
            for h in range(H_LOC):
                e_t = e_ts[h]
                q0 = h * S
                at_ps = psr.tile([128, CHUNK], F32, tag=qtag, name="at_ps")
                rs_ps = psr.tile([128, CHUNK], F32, tag=qtag, name="rs_ps")
                for term in range(2):
                    v_t = (v8_t, vr_t)[term]
                    for ki in range(nki):
                        trim = 128 * ki
                        nc.tensor.matmul(
                            at_ps[:, trim:CHUNK],
                            v_t[:, ki * F_LOC + h * HD:
                                ki * F_LOC + (h + 1) * HD],
                            e_t[:, ki * CHUNK + trim:(ki + 1) * CHUNK],
                            start=(term == 0 and ki == 0),
                            stop=(term == 1 and ki == nki - 1),
                        )
                for ki in range(nki):
                    trim = 128 * ki
                    nc.tensor.matmul(
                        rs_ps[:, trim:CHUNK],
                        ones16[:],
                        e_t[:, ki * CHUNK + trim:(ki + 1) * CHUNK],
                        start=(ki == 0), stop=(ki == nki - 1),
                    )
                ri_t = ripool.tile([128, CHUNK], F32, tag="ri", name="ri_t")
                nc.vector.reciprocal(ri_t[:], rs_ps[:])
                nc.vector.tensor_tensor(
                    attnT_t[:, q0: q0 + CHUNK],
                    at_ps[:], ri_t[:], MULT)

        def attn_scores(c, acts):
            """Scores + exp for one 512-query chunk (c>=1), both heads."""
            qT_t, kT_t, v8_t, vr_t, attnT_t = acts
            nki = 4 * c + 4
            shift = shift_t[c]
            e_ts = []
            for h in range(H_LOC):
                e_t = xkpool.tile([128, KT * CHUNK], FP8E4, tag="e8",
                                  bufs=5, name="e_t")
                e_ts.append(e_t)
                q0 = h * S + c * CHUNK
                # scoresT blocks + exp (transposed layout: [k, q]), bf16 PE
                for ki in range(nki):
                    r = ki - 4 * c
                    trim = 128 * r if r > 0 else 0
                    ncol = CHUNK - trim
                    diag = ki >= 4 * c
                    s_ps = psa.tile([128, CHUNK], F32, tag=qtag, name="s_ps")
                    nc.tensor.matmul(
                        s_ps[:, 0:ncol],
                        kT_t[:, h * S + ki * 128: h * S + (ki + 1) * 128],
                        qT_t[:, q0 + trim: q0 + CHUNK],
                        start=True, stop=not diag,
                    )
                    if diag:  # diagonal 128x128 needs the causal mask
                        nc.tensor.matmul(
                            s_ps[:, 0:128], id128[:], maskT[:],
                            start=False, stop=True,
                        )
                    nc.scalar.activation(
                        e_t[:, ki * CHUNK + trim:(ki + 1) * CHUNK],
                        s_ps[:, 0:ncol], EXP, bias=shift[:],
                        scale=INV_SQRT_HD)
                # zero the sub-diagonal strips of the odd diagonal blocks
                # (they sit inside the DoubleRow pair's shared column span)
                nc.gpsimd.memset(
                    e_t[:, (4 * c + 1) * CHUNK:(4 * c + 1) * CHUNK + 128],
                    0.0)
                nc.gpsimd.memset(
                    e_t[:, (4 * c + 3) * CHUNK + 256:
                        (4 * c + 3) * CHUNK + 384],
                    0.0)
            return e_ts

        def attn_pv(c, acts, e_ts):
            """PV + rowsum + normalize for one chunk (c>=1), both heads."""
            qT_t, kT_t, v8_t, vr_t, attnT_t = acts
            nki = 4 * c + 4
            npair = nki // 2
            v8k = v8_t[:].rearrange("p (k f) -> p k f", k=TT)
            vrk = vr_t[:].rearrange("p (k f) -> p k f", k=TT)
            for h in range(H_LOC):
                ek = kview(e_ts[h])
                q0 = h * S + c * CHUNK
                # PV (2 terms: v8, vr) and rowsum over key-block pairs
                hs = slice(h * HD, (h + 1) * HD)
                at_ps = psr.tile([128, CHUNK], F32, tag=qtag, name="at_ps")
                rs_ps = psr.tile([128, CHUNK], F32, tag=qtag, name="rs_ps")
                n = 2 * npair
                i = 0
                for term in range(2):
                    vk = (v8k, vrk)[term]
                    for kp in range(npair):
                        ptrim = 256 if kp == npair - 1 else 0
                        nc.tensor.matmul(
                            at_ps[:, ptrim:CHUNK],
                            vk[:, 2 * kp:2 * kp + 2, hs],
                            ek[:, 2 * kp:2 * kp + 2, ptrim:CHUNK],
                            start=(i == 0), stop=(i == n - 1),
                            perf_mode=DRM,
                        )
                        i += 1
                for kp in range(npair):
                    ptrim = 256 if kp == npair - 1 else 0
                    nc.tensor.matmul(
                        rs_ps[:, ptrim:CHUNK],
                        ones8[:].rearrange("p (k f) -> p k f", k=2),
                        ek[:, 2 * kp:2 * kp + 2, ptrim:CHUNK],
                        start=(kp == 0), stop=(kp == npair - 1),
                        perf_mode=DRM,
                    )
                ri_t = ripool.tile([128, CHUNK], F32, tag="ri", name="ri_t")
                nc.vector.reciprocal(ri_t[:], rs_ps[:])
                nc.vector.tensor_tensor(
                    attnT_t[:, q0: q0 + CHUNK],
                    at_ps[:], ri_t[:], MULT)

        def outproj_chunk(b, c, acts, wo_t):
            """Output projection + y writeback for one chunk's tokens."""
            attnT_t = acts[4]
            for t4 in range(CHUNK // 128):
                tt = c * (CHUNK // 128) + t4
                for oc in range(D // CHUNK):
                    o_ps = pso.tile([128, CHUNK], F32, tag="pso",
                                    name="o_ps")
                    for h in range(H_LOC):
                        nc.tensor.matmul(
                            o_ps[:],
                            attnT_t[:, h * S + tt * 128:
                                    h * S + (tt + 1) * 128],
                            wo_t[:, h * D + oc * CHUNK:
                                 h * D + (oc + 1) * CHUNK],
                            start=(h == 0), stop=(h == H_LOC - 1),
                        )
                    y_t = ypool.tile([128, CHUNK], BF16, tag="y", name="y_t")
                    nc.vector.tensor_copy(y_t[:], o_ps[:])
                    row0 = b * S + tt * 128
                    nc.sync.dma_start(
                        y_d.ap()[row0:row0 + 128,
                                 oc * CHUNK:(oc + 1) * CHUNK], y_t[:])

        def new_acts():
            qT_t = actpool.tile([128, H_LOC * S], BF16, tag="act",
                                bufs=4, name="qT_t")
            kT_t = actpool.tile([128, H_LOC * S], BF16, tag="act",
                                bufs=4, name="kT_t")
            v8_t = actpool.tile([128, TT * F_LOC], FP8E4, tag="actv",
                                bufs=2, name="v8_t")
            vr_t = actpool.tile([128, TT * F_LOC], FP8E4, tag="actvr",
                                bufs=2, name="vr_t")
            a8_t = actpool.tile([128, H_LOC * S], FP8E4, tag="attn8",
                                bufs=2, name="a8_t")
            ar_t = actpool.tile([128, H_LOC * S], FP8E4, tag="attnr",
                                bufs=2, name="ar_t")
            return (qT_t, kT_t, v8_t, vr_t, (a8_t, ar_t))

        def load_wo():
            # woT [F_LOC, D] -> [128, H_LOC*D] fp8 value/residual pair;
            # deferred so the DMA queue prioritizes x chunks during warmup
            ts = []
            for nm, dram in (("wo8", wo8_d), ("wor", wor_d)):
                wo_t = wpool.tile([128, H_LOC * D], FP8E4, tag=nm, name=nm)
                nc.sync.dma_start(
                    wo_t[:].rearrange("p (h f) -> p h f", h=H_LOC),
                    dram.ap().rearrange("(h p) f -> p h f", p=128),
                )
                ts.append(wo_t)
            return ts

        def attn_sc(c, acts):
            return attn0_scores(acts) if c == 0 else attn_scores(c, acts)

        def attn_fin(c, acts, e_ts):
            if c == 0:
                attn0_pv(acts, e_ts)
            else:
                attn_pv(c, acts, e_ts)

        def emit_body(first_iter=True):
            wo_t = None
            loaded = {}
            if first_iter:
                loaded[(0, 0)] = x_first
                loaded[(0, 1)] = x_second
            for b in range(B):
                acts = new_acts()
                for c in range(NCH):
                    if (b, c) not in loaded:
                        loaded[(b, c)] = load_x(b, c)
                    # prefetch the next chunk so its DMA overlaps compute
                    if c + 1 < NCH and (b, c + 1) not in loaded:
                        loaded[(b, c + 1)] = load_x(b, c + 1)
                    if wo_t is None and c == NCH - 1:
                        wo_t = load_wo()
                    qkv_chunk(loaded.pop((b, c)), c, acts)
                if b + 1 < B:
                    # next batch's first chunk lands during this attn phase
                    loaded[(b + 1, 0)] = load_x(b + 1, 0)
                # software pipeline: outproj(c-1) sits between scores(c)
                # and PV(c) so its PSUM drain and the attnT normalize chain
                # overlap PE score work instead of stalling it
                prev = None
                for c in range(NCH):
                    e_ts = attn_sc(c, acts)
                    if prev is not None:
                        outproj_chunk(b, prev, acts, wo_t)
                    attn_fin(c, acts, e_ts)
                    prev = c
                outproj_chunk(b, prev, acts, wo_t)

        if reps is None:
            emit_body()
        else:
            with tc.For_i(0, reps, 1):
                emit_body(first_iter=False)

        pools = [pso, psq, ripool, ypool, actpool, xkpool,
                 wpool, cpool]
        seen = set()
        for p in pools:
            if id(p) not in seen:
                seen.add(id(p))
                p.release()

    nc.compile()
    return nc


def _get_nc(reps=None):
    key = ("nc", reps)
    if key not in _CACHE:
        _CACHE[key] = _build(reps)
    return _CACHE[key]


def _fp8(a):
    import ml_dtypes
    return np.ascontiguousarray(a).astype(ml_dtypes.float8_e4m3)


def _bf16(a):
    import ml_dtypes
    return np.ascontiguousarray(a).astype(ml_dtypes.bfloat16)


def make_in_maps(x, wq, bq, wk, bk, wv, bv, wo):
    x = np.asarray(x, dtype=np.float32)
    xT = np.ascontiguousarray(x.reshape(B * S, D).T)  # [D, B*S]
    x8T = _fp8(xT)
    xrT = _fp8(xT - x8T.astype(np.float32))
    # pack value+residual interleaved by k-tile: rows (k, two, p)
    xiT = np.empty((KT, 2, 128, B * S), dtype=x8T.dtype)
    xiT[:, 0] = x8T.reshape(KT, 128, B * S)
    xiT[:, 1] = xrT.reshape(KT, 128, B * S)
    xiT = np.ascontiguousarray(xiT.reshape(2 * D, B * S))

    in_maps = []
    for i in range(N_CORES):
        fs = slice(i * F_LOC, (i + 1) * F_LOC)
        m = {
            "xiT": xiT,
            "woT": _bf16(np.asarray(wo)[:, fs].T),
            "bq2": np.ascontiguousarray(
                np.asarray(bq)[fs].reshape(H_LOC, HD).T.astype(np.float32)),
            "bk2": np.ascontiguousarray(
                np.asarray(bk)[fs].reshape(H_LOC, HD).T.astype(np.float32)),
            "ones": _fp8(np.full((128, 256), W_SCALE, dtype=np.float32)),
            "id128": _bf16(np.eye(128, dtype=np.float32)),
        }
        for nm, w in (("wq", wq), ("wk", wk), ("wv", wv)):
            wT = W_SCALE * np.asarray(w, dtype=np.float32)[fs, :].T
            w8 = _fp8(wT)
            m[nm + "8T"] = w8
            m[nm + "rT"] = _fp8(wT - w8.astype(np.float32))
        in_maps.append(m)
    return in_maps


def kernel(x, wq, bq, wk, bk, wv, bv, wo, bo):
    from concourse.bass_utils import run_bass_kernel_spmd

    nc = _get_nc()
    in_maps = make_in_maps(x, wq, bq, wk, bk, wv, bv, wo)
    res = run_bass_kernel_spmd(nc, in_maps, core_ids=list(range(N_CORES)),
                               trace=False)
    y = np.zeros((B * S, D), dtype=np.float32)
    for i in range(N_CORES):
        y += res.results[i]["y"].astype(np.float32)
    # v bias folded through the output projection, plus bo
    y += (np.asarray(wo, dtype=np.float32) @ np.asarray(bv, np.float32)
          + np.asarray(bo, np.float32))[None, :]
    return y.reshape(B, S, D)
